# revision 8
# baseline (speedup 1.0000x reference)
"""Supervised-contrastive loss on 8 TRN2 NeuronCores — v3.

Math (identical to the reference):
    s_ij  = cosine similarity of feature rows i, j
    E_ij  = exp(s_ij / tau)
    neg_i = sum_j E_ij * (1 - mask_ij)          (mask = same-class, incl diag)
    loss  = sum_{i, same-class j != i} [ln(E_ij + neg_i) - s_ij/tau] / p_i
            ---------------------------------------------------------------
                                     sum_i p_i

Key ideas:
  * Rows are SORTED BY CLASS on the host, so every same-class pair (i, j)
    satisfies |i - j| < 128.  All mask work and the ln() pass then touch
    only a W=384-column diagonal band instead of the full 4096 columns.
  * The GEMM runs in fp8 (e4m3, x64 pre-scale) with DoubleRow perf mode:
    256-deep contraction per matmul, half the matmul count of bf16.
  * Each core receives a column-ROTATED copy of fnT8 (own block at local
    columns [512, 1024)), which makes the program core-independent; the
    band wrap-around columns carry zero masks, so they only contribute
    ln(neg) terms that the host subtracts in closed form.
  * exp and ln share one ACT table set (natural_log_exp_and_others), so
    the per-row-tile ln can interleave with exp at zero switch cost.
  * DMA params are laid out per-partition-contiguous (one DIRECT2D
    descriptor-gen each) and spread across four engine issue queues.
  * The band lives in local half 0, so the masked-band products run
    during pass 1; only neg/ln trail the half-1 row sums.

Device outputs per row: lnsum_i (band ln-sum) and neg_i.
Host postprocess (O(N*D)):
    A_i  = lnsum_i - (W - (p_i - 1)) * ln(neg_i)   -> sum_masked ln(E+neg)
    B_i  = (fnq_i . g(class_i) - |fnq_i|^2) / tau  -> sum_masked s/tau
    loss = sum((A - B)/p) / sum(p)
"""

import numpy as np
import ml_dtypes

TAU = 0.1
N, D = 4096, 512
NCORES = 8
ROWS = N // NCORES          # 512 rows per core
IT = ROWS // 128            # 4 partition tiles per core
W = 384                     # band width (max class size 61 << 129 bound)
PAD = 128                   # band left-overhang
S8 = 64.0                   # fp8 pre-scale

_CACHE = {}


def _build_nc():
    import concourse.tile as tile
    import concourse.mybir as mybir
    from concourse import bacc

    dt = mybir.dt
    AF = mybir.ActivationFunctionType
    ALU = mybir.AluOpType
    AX = mybir.AxisListType
    PM = mybir.MatmulPerfMode

    # Force Exp AND Ln to resolve to the one table set that holds both, so
    # a single ACT_TABLE_LOAD serves the whole kernel.  Entries keep their
    # original indices (ids index act_info.json) — we only blank the
    # Exp/Ln membership of the competing sets during this build.
    orig_get = bacc.get_activation_tables

    def patched(arch):
        out = {}
        for name, fns in orig_get(arch).items():
            if name != "natural_log_exp_and_others" and (
                AF.Exp in fns or AF.Ln in fns
            ):
                fns = {f for f in fns if f not in (AF.Exp, AF.Ln)}
            out[name] = fns
        return out

    bacc.get_activation_tables = patched
    try:
        nc = bacc.Bacc(None)
        # fn{kp}{h}: per-partition [i(2), local_col(2048)] fp8, contiguous
        # 4KB per partition; contraction row of (kp, i, p) = kp*256+i*128+p.
        # half-0 params split into 1024-col quarters (earlier first MM);
        # half-1 stays whole.
        fn_q = [
            [
                nc.declare_dram_parameter(
                    f"fnq{kp}{piece}", [128, 2, 1024], dt.float8e4, isOutput=False)
                for piece in range(2)
            ]
            for kp in range(2)
        ]
        fn_h1 = [
            nc.declare_dram_parameter(
                f"fn{kp}1", [128, 2, 2048], dt.float8e4, isOutput=False)
            for kp in range(2)
        ]
        m1 = nc.declare_dram_parameter("m1", [128, IT * W], dt.float8e4, isOutput=False)
        ln_out = nc.declare_dram_parameter("ln_out", [128, IT], dt.float32, isOutput=True)
        neg_out = nc.declare_dram_parameter("neg_out", [128, IT], dt.float32, isOutput=True)

        with tile.TileContext(nc) as tc:
            with (
                tc.tile_pool(name="persist", bufs=1) as persist,
                tc.tile_pool(name="psum", bufs=2, space="PSUM") as psum,
                tc.tile_pool(name="acc", bufs=2) as accp,
                tc.tile_pool(name="band", bufs=2) as bandp,
                tc.tile_pool(name="outp", bufs=1) as outp,
            ):
                # ---- persistent SBUF ----
                FN = [
                    [
                        persist.tile([128, 2, 2048], dt.float8e4,
                                     name=f"fn{kp}{h}", tag=f"fn{kp}{h}")
                        for h in range(2)
                    ]
                    for kp in range(2)
                ]
                M1s = persist.tile([128, IT * W], dt.float8e4, tag="m1")
                E = [persist.tile([128, N], dt.bfloat16, name=f"e{it}", tag=f"e{it}")
                     for it in range(IT)]
                lnout_sb = outp.tile([128, IT], dt.float32, tag="lnout")
                negout_sb = outp.tile([128, IT], dt.float32, tag="negout")

                # ---- DMA: one contiguous transfer per param, spread over
                # four engine issue queues so descriptor-gen overlaps.
                with tc.high_priority():
                    nc.sync.dma_start(FN[0][0][:, :, 0:1024], fn_q[0][0][:])
                    nc.sync.dma_start(FN[1][0][:, :, 0:1024], fn_q[1][0][:])
                nc.sync.dma_start(FN[0][0][:, :, 1024:2048], fn_q[0][1][:])
                nc.sync.dma_start(FN[1][0][:, :, 1024:2048], fn_q[1][1][:])
                nc.sync.dma_start(FN[0][1][:], fn_h1[0][:])
                nc.sync.dma_start(FN[1][1][:], fn_h1[1][:])
                nc.gpsimd.dma_start(M1s[:], m1[:])

                # hoisted stationary APs (4 matmuls reuse each)
                lhsT = [
                    [FN[kp][0][:, :, 512 + it * 128: 512 + it * 128 + 128]
                     for it in range(IT)]
                    for kp in range(2)
                ]

                EXP_SCALE = 1.0 / (TAU * S8 * S8)

                def gemm_exp(it, h, split_exp=False):
                    Sh = psum.tile([128, 2048], dt.float32, tag="S")
                    qorder = [1, 0, 2, 3] if h == 0 else [0, 1, 2, 3]
                    for kp in range(2):
                        for q in qorder:
                            nc.tensor.matmul(
                                Sh[:, q * 512:(q + 1) * 512],
                                lhsT[kp][it],
                                FN[kp][h][:, :, q * 512:(q + 1) * 512],
                                start=(kp == 0),
                                stop=(kp == 1),
                                perf_mode=PM.DoubleRow,
                            )
                    base = h * 2048
                    if not split_exp:
                        nc.scalar.activation(
                            E[it][:, base:base + 2048], Sh[:], AF.Exp,
                            scale=EXP_SCALE)
                        return None
                    # last tile: split + accumulate on ACT for the shortest
                    # end-of-kernel dependency chain.
                    acc = accp.tile([128, 2], dt.float32, tag="expacc")
                    nc.scalar.activation(
                        E[it][:, base:base + 1024], Sh[:, 0:1024], AF.Exp,
                        scale=EXP_SCALE, accum_out=acc[:, 0:1])
                    nc.scalar.activation(
                        E[it][:, base + 1024:base + 2048], Sh[:, 1024:2048],
                        AF.Exp, scale=EXP_SCALE, accum_out=acc[:, 1:2])
                    return acc

                # ---- pass 1: local half 0 (contains the whole band) ----
                # band = local cols [384 + it*128, +W); masked products can
                # run as soon as this tile's half-0 exp lands.
                band_st = []

                def band_mul(it):
                    Eb = E[it][:, 384 + it * 128: 384 + it * 128 + W]
                    rsEM_t = accp.tile([128, 1], dt.float32, tag=f"rsem_{it}")
                    EM1 = bandp.tile([128, W], dt.bfloat16, tag=f"em1_{it}")
                    nc.vector.scalar_tensor_tensor(
                        EM1[:], Eb, 1.0, M1s[:, it * W:(it + 1) * W],
                        ALU.mult, ALU.mult, accum_out=rsEM_t[:],
                    )
                    band_st.append((rsEM_t, EM1))

                rsEh = [accp.tile([128, 2], dt.float32, name=f"rseh_{it}",
                                  tag=f"rseh_{it}") for it in range(IT)]
                for it in range(IT):
                    gemm_exp(it, 0)
                    band_mul(it)
                    nc.vector.tensor_reduce(
                        rsEh[it][:, 0:1], E[it][:, 0:2048], AX.X, ALU.add)

                # ---- pass 2: half 1; neg + band ln trail per tile ----
                negs = []

                def neg_calc(it, exp_acc):
                    rsEM_t, EM1 = band_st[it]
                    rsE_t = accp.tile([128, 1], dt.float32, tag="rse_t")
                    neg_t = accp.tile([128, 1], dt.float32, tag=f"neg_{it}")
                    if exp_acc is None:
                        nc.vector.tensor_reduce(
                            rsEh[it][:, 1:2], E[it][:, 2048:4096], AX.X, ALU.add)
                    else:
                        nc.vector.tensor_reduce(
                            rsEh[it][:, 1:2], exp_acc[:], AX.X, ALU.add)
                    nc.vector.tensor_reduce(rsE_t[:], rsEh[it][:], AX.X, ALU.add)
                    nc.vector.tensor_sub(neg_t[:], rsE_t[:], rsEM_t[:])
                    nc.vector.tensor_copy(negout_sb[:, it:it + 1], neg_t[:])
                    negs.append(neg_t)

                def band_ln(it, on_act):
                    Lb = bandp.tile([128, W], dt.bfloat16, tag=f"lb_{it}")
                    if on_act:
                        nc.scalar.activation(
                            Lb[:], band_st[it][1][:], AF.Ln,
                            bias=negs[it][:, 0:1],
                            accum_out=lnout_sb[:, it:it + 1],
                        )
                    else:
                        nc.scalar.activation(
                            Lb[:], band_st[it][1][:], AF.Ln,
                            bias=negs[it][:, 0:1],
                        )
                        nc.vector.tensor_reduce(
                            lnout_sb[:, it:it + 1], Lb[:], AX.X, ALU.add)

                for it in range(IT):
                    acc = gemm_exp(it, 1, split_exp=(it == IT - 1))
                    neg_calc(it, acc)
                    if it >= 1:
                        band_ln(it - 1, on_act=False)
                nc.gpsimd.dma_start(neg_out[:], negout_sb[:])
                band_ln(IT - 1, on_act=True)

                nc.sync.dma_start(ln_out[:], lnout_sb[:])

        nc.finalize()
    finally:
        bacc.get_activation_tables = orig_get
    return nc


def _get_nc():
    if "nc" not in _CACHE:
        _CACHE["nc"] = _build_nc()
    return _CACHE["nc"]


def _host_prep(features, targets):
    f8t = ml_dtypes.float8_e4m3
    f = np.asarray(features, np.float32)
    t = np.asarray(targets).astype(np.int64)

    perm = np.argsort(t, kind="stable")
    fs, ts = f[perm], t[perm]
    rnorm = 1.0 / np.sqrt((fs.astype(np.float64) ** 2).sum(1))
    fn = (fs * rnorm[:, None].astype(np.float32)).astype(np.float32)
    fn8 = (fn * S8).astype(f8t)                     # [N, D] fp8 values
    fnT8 = np.ascontiguousarray(fn8.T)              # [D, N]

    in_maps = []
    for c in range(NCORES):
        roll = np.roll(fnT8, 512 - c * 512, axis=1)     # local col l = global (c*512-512+l) % N
        a = roll.reshape(2, 2, 128, N)                  # [kp, i, p, l]
        im = {}
        for kp in range(2):
            for piece in range(2):
                im[f"fnq{kp}{piece}"] = np.ascontiguousarray(
                    a[kp, :, :, piece * 1024:(piece + 1) * 1024].transpose(1, 0, 2))
            im[f"fn{kp}1"] = np.ascontiguousarray(
                a[kp, :, :, 2048:4096].transpose(1, 0, 2))
        # band masks, local band cols of row tile it: global (R0 - PAD + j) % N
        it_i = np.arange(IT)
        R0 = c * 512 + it_i * 128
        rows = R0[:, None] + np.arange(128)[None, :]            # [IT, p]
        g = (R0[:, None] - PAD + np.arange(W)[None, :]) % N     # [IT, j]
        m1 = (ts[rows][:, :, None] == ts[g][:, None, :])        # [IT, p, j]
        im["m1"] = np.ascontiguousarray(
            m1.transpose(1, 0, 2).reshape(128, IT * W).astype(f8t))
        in_maps.append(im)
    return (fn8, ts), in_maps


def _band_covered(ts):
    """Every same-class pair must fall inside the band (guaranteed for any
    remotely Poisson-like class distribution; checked for safety)."""
    cls, counts = np.unique(ts, return_counts=True)
    starts = np.zeros(len(cls) + 1, np.int64)
    starts[1:] = np.cumsum(counts)
    idx = np.searchsorted(cls, ts)
    row_lo, row_hi = starts[idx], starts[idx] + counts[idx]
    R0 = (np.arange(N) // 128) * 128
    return bool(((row_lo >= R0 - PAD) & (row_hi <= R0 - PAD + W)).all())


def _host_post(fn8, ts, lnsum_rows, neg_rows):
    cls, counts = np.unique(ts, return_counts=True)
    idx = np.searchsorted(cls, ts)
    p = counts[idx].astype(np.float64)
    Eii = np.exp((fn8.astype(np.float64) ** 2).sum(1) / (TAU * S8 * S8))
    A = (lnsum_rows - (W - p) * np.log(neg_rows)
         - np.log(Eii + neg_rows))
    fnq = fn8.astype(np.float64) / S8
    g = np.zeros((len(cls), D), np.float64)
    np.add.at(g, idx, fnq)
    B = ((fnq * g[idx]).sum(1) - (fnq ** 2).sum(1)) / TAU
    loss = ((A - B) / p).sum() / p.sum()
    return np.float32(loss)


def _rows_from_out(per_core_outs):
    lnsum = np.empty(N, np.float64)
    neg = np.empty(N, np.float64)
    for c, out in enumerate(per_core_outs):
        ln = np.asarray(out["ln_out"], np.float64)     # [128, IT]
        ng = np.asarray(out["neg_out"], np.float64)
        lnsum[c * ROWS:(c + 1) * ROWS] = ln.T.reshape(ROWS)
        neg[c * ROWS:(c + 1) * ROWS] = ng.T.reshape(ROWS)
    return lnsum, neg


def _run(in_maps, trace=False):
    from concourse.bass_utils import run_bass_kernel_spmd
    nc = _get_nc()
    return run_bass_kernel_spmd(
        nc, in_maps, core_ids=list(range(NCORES)), trace=trace,
    )


def _numpy_fallback(features, targets):
    f = np.asarray(features, np.float64)
    t = np.asarray(targets).astype(np.int64)
    sim = f @ f.T
    nrm = np.sqrt((f ** 2).sum(1))
    nm = np.maximum(nrm[:, None] * nrm[None, :], 1e-8)
    E = np.exp(sim / nm / TAU)
    mask = (t[None, :] == t[:, None])
    np.fill_diagonal(E, 0.0)
    negv = (E * ~mask).sum(1)
    p = mask.sum(1).astype(np.float64)
    with np.errstate(divide="ignore"):
        lm = np.where(mask & (E > 0), np.log(E / (E + negv[:, None])), 0.0)
    return np.float32(-(lm / p[:, None]).sum() / p.sum())


def kernel(features, targets):
    (fn8, ts), in_maps = _host_prep(features, targets)
    if not _band_covered(ts):
        return _numpy_fallback(features, targets)
    res = _run(in_maps, trace=False)
    lnsum_rows, neg_rows = _rows_from_out(res.results)
    return _host_post(fn8, ts, lnsum_rows, neg_rows)


# revision 9
# speedup vs baseline: 1.0629x; 1.0629x over previous
"""Supervised-contrastive loss on 8 TRN2 NeuronCores — v3.

Math (identical to the reference):
    s_ij  = cosine similarity of feature rows i, j
    E_ij  = exp(s_ij / tau)
    neg_i = sum_j E_ij * (1 - mask_ij)          (mask = same-class, incl diag)
    loss  = sum_{i, same-class j != i} [ln(E_ij + neg_i) - s_ij/tau] / p_i
            ---------------------------------------------------------------
                                     sum_i p_i

Key ideas:
  * Rows are SORTED BY CLASS on the host, so every same-class pair (i, j)
    satisfies |i - j| < 128.  All mask work and the ln() pass then touch
    only a W=384-column diagonal band instead of the full 4096 columns.
  * The GEMM runs in fp8 (e4m3, x64 pre-scale) with DoubleRow perf mode:
    256-deep contraction per matmul, half the matmul count of bf16.
  * Each core receives a column-ROTATED copy of fnT8 (own block at local
    columns [512, 1024)), which makes the program core-independent; the
    band wrap-around columns carry zero masks, so they only contribute
    ln(neg) terms that the host subtracts in closed form.
  * exp and ln share one ACT table set (natural_log_exp_and_others), so
    the per-row-tile ln can interleave with exp at zero switch cost.
  * DMA params are laid out per-partition-contiguous (one DIRECT2D
    descriptor-gen each) and spread across four engine issue queues.
  * The band lives in local half 0, so the masked-band products run
    during pass 1; only neg/ln trail the half-1 row sums.

Device outputs per row: lnsum_i (band ln-sum) and neg_i.
Host postprocess (O(N*D)):
    A_i  = lnsum_i - (W - (p_i - 1)) * ln(neg_i)   -> sum_masked ln(E+neg)
    B_i  = (fnq_i . g(class_i) - |fnq_i|^2) / tau  -> sum_masked s/tau
    loss = sum((A - B)/p) / sum(p)
"""

import numpy as np
import ml_dtypes

TAU = 0.1
N, D = 4096, 512
NCORES = 8
ROWS = N // NCORES          # 512 rows per core
IT = ROWS // 128            # 4 partition tiles per core
W = 384                     # band width (max class size 61 << 129 bound)
PAD = 128                   # band left-overhang
S8 = 64.0                   # fp8 pre-scale

_CACHE = {}


def _build_nc():
    import concourse.tile as tile
    import concourse.mybir as mybir
    from concourse import bacc

    dt = mybir.dt
    AF = mybir.ActivationFunctionType
    ALU = mybir.AluOpType
    AX = mybir.AxisListType
    PM = mybir.MatmulPerfMode

    # Force Exp AND Ln to resolve to the one table set that holds both, so
    # a single ACT_TABLE_LOAD serves the whole kernel.  Entries keep their
    # original indices (ids index act_info.json) — we only blank the
    # Exp/Ln membership of the competing sets during this build.
    orig_get = bacc.get_activation_tables

    def patched(arch):
        out = {}
        for name, fns in orig_get(arch).items():
            if name != "natural_log_exp_and_others" and (
                AF.Exp in fns or AF.Ln in fns
            ):
                fns = {f for f in fns if f not in (AF.Exp, AF.Ln)}
            out[name] = fns
        return out

    bacc.get_activation_tables = patched
    try:
        nc = bacc.Bacc(None)
        # fn{kp}{h}: per-partition [i(2), local_col(2048)] fp8, contiguous
        # 4KB per partition; contraction row of (kp, i, p) = kp*256+i*128+p.
        # half-0 params split into 1024-col quarters (earlier first MM);
        # half-1 stays whole.
        fn_q = [
            [
                nc.declare_dram_parameter(
                    f"fnq{kp}{piece}", [128, 2, 1024], dt.float8e4, isOutput=False)
                for piece in range(2)
            ]
            for kp in range(2)
        ]
        fn_h1 = [
            nc.declare_dram_parameter(
                f"fn{kp}1", [128, 2, 2048], dt.float8e4, isOutput=False)
            for kp in range(2)
        ]
        m1 = nc.declare_dram_parameter("m1", [128, IT * W], dt.float8e4, isOutput=False)
        ln_out = nc.declare_dram_parameter("ln_out", [128, IT], dt.float32, isOutput=True)
        neg_out = nc.declare_dram_parameter("neg_out", [128, IT], dt.float32, isOutput=True)

        with tile.TileContext(nc) as tc:
            with (
                tc.tile_pool(name="persist", bufs=1) as persist,
                tc.tile_pool(name="psum", bufs=2, space="PSUM") as psum,
                tc.tile_pool(name="acc", bufs=2) as accp,
                tc.tile_pool(name="band", bufs=2) as bandp,
                tc.tile_pool(name="outp", bufs=1) as outp,
            ):
                # ---- persistent SBUF ----
                FN = [
                    [
                        persist.tile([128, 2, 2048], dt.float8e4,
                                     name=f"fn{kp}{h}", tag=f"fn{kp}{h}")
                        for h in range(2)
                    ]
                    for kp in range(2)
                ]
                M1s = persist.tile([128, IT * W], dt.float8e4, tag="m1")
                rsE2 = [accp.tile([128, 3], dt.float32, name=f"rse2_{it}",
                                  tag=f"rse2_{it}") for it in range(IT)]
                E = [persist.tile([128, N], dt.bfloat16, name=f"e{it}", tag=f"e{it}")
                     for it in range(IT)]
                lnout_sb = outp.tile([128, IT], dt.float32, tag="lnout")
                negout_sb = outp.tile([128, IT], dt.float32, tag="negout")

                # ---- DMA: one contiguous transfer per param, spread over
                # four engine issue queues so descriptor-gen overlaps.
                with tc.high_priority():
                    nc.sync.dma_start(FN[0][0][:, :, 0:1024], fn_q[0][0][:])
                    nc.sync.dma_start(FN[1][0][:, :, 0:1024], fn_q[1][0][:])
                nc.sync.dma_start(FN[0][0][:, :, 1024:2048], fn_q[0][1][:])
                nc.sync.dma_start(FN[1][0][:, :, 1024:2048], fn_q[1][1][:])
                nc.sync.dma_start(FN[0][1][:], fn_h1[0][:])
                nc.sync.dma_start(FN[1][1][:], fn_h1[1][:])
                nc.gpsimd.dma_start(M1s[:], m1[:])

                # hoisted stationary APs (4 matmuls reuse each)
                lhsT = [
                    [FN[kp][0][:, :, 512 + it * 128: 512 + it * 128 + 128]
                     for it in range(IT)]
                    for kp in range(2)
                ]

                EXP_SCALE = 1.0 / (TAU * S8 * S8)

                def gemm_exp(it, h, split_exp=False):
                    Sh = psum.tile([128, 2048], dt.float32, tag="S")
                    qorder = [1, 0, 2, 3] if h == 0 else [0, 1, 2, 3]
                    for kp in range(2):
                        for q in qorder:
                            nc.tensor.matmul(
                                Sh[:, q * 512:(q + 1) * 512],
                                lhsT[kp][it],
                                FN[kp][h][:, :, q * 512:(q + 1) * 512],
                                start=(kp == 0),
                                stop=(kp == 1),
                                perf_mode=PM.DoubleRow,
                            )
                    base = h * 2048
                    if not split_exp:
                        nc.scalar.activation(
                            E[it][:, base:base + 2048], Sh[:], AF.Exp,
                            scale=EXP_SCALE, accum_out=rsE2[it][:, h:h + 1])
                        return 2
                    # last tile: split the final exp so the end-of-kernel
                    # dependency chain is one 1024-wide exp shorter.
                    nc.scalar.activation(
                        E[it][:, base:base + 1024], Sh[:, 0:1024], AF.Exp,
                        scale=EXP_SCALE, accum_out=rsE2[it][:, 1:2])
                    nc.scalar.activation(
                        E[it][:, base + 1024:base + 2048], Sh[:, 1024:2048],
                        AF.Exp, scale=EXP_SCALE, accum_out=rsE2[it][:, 2:3])
                    return 3

                # ---- pass 1: local half 0 (contains the whole band) ----
                # band = local cols [384 + it*128, +W); masked products can
                # run as soon as this tile's half-0 exp lands.
                band_st = []

                def band_mul(it):
                    Eb = E[it][:, 384 + it * 128: 384 + it * 128 + W]
                    rsEM_t = accp.tile([128, 1], dt.float32, tag=f"rsem_{it}")
                    EM1 = bandp.tile([128, W], dt.bfloat16, tag=f"em1_{it}")
                    nc.vector.scalar_tensor_tensor(
                        EM1[:], Eb, 1.0, M1s[:, it * W:(it + 1) * W],
                        ALU.mult, ALU.mult, accum_out=rsEM_t[:],
                    )
                    band_st.append((rsEM_t, EM1))

                for it in range(IT):
                    gemm_exp(it, 0)
                    band_mul(it)

                # ---- pass 2: half 1; neg + band ln trail per tile ----
                negs = []

                def neg_calc(it, ncols):
                    rsEM_t, EM1 = band_st[it]
                    rsE_t = accp.tile([128, 1], dt.float32, tag="rse_t")
                    neg_t = accp.tile([128, 1], dt.float32, tag=f"neg_{it}")
                    nc.vector.tensor_reduce(
                        rsE_t[:], rsE2[it][:, 0:ncols], AX.X, ALU.add)
                    nc.vector.tensor_sub(neg_t[:], rsE_t[:], rsEM_t[:])
                    nc.vector.tensor_copy(negout_sb[:, it:it + 1], neg_t[:])
                    negs.append(neg_t)

                def band_ln(it):
                    Lb = bandp.tile([128, W], dt.bfloat16, tag=f"lb_{it}")
                    nc.scalar.activation(
                        Lb[:], band_st[it][1][:], AF.Ln,
                        bias=negs[it][:, 0:1],
                        accum_out=lnout_sb[:, it:it + 1],
                    )

                for it in range(IT):
                    ncols = gemm_exp(it, 1, split_exp=(it == IT - 1))
                    neg_calc(it, ncols)
                    if it >= 1:
                        band_ln(it - 1)
                nc.gpsimd.dma_start(neg_out[:], negout_sb[:])
                band_ln(IT - 1)

                nc.sync.dma_start(ln_out[:], lnout_sb[:])

        nc.finalize()
    finally:
        bacc.get_activation_tables = orig_get
    return nc


def _get_nc():
    if "nc" not in _CACHE:
        _CACHE["nc"] = _build_nc()
    return _CACHE["nc"]


def _host_prep(features, targets):
    f8t = ml_dtypes.float8_e4m3
    f = np.asarray(features, np.float32)
    t = np.asarray(targets).astype(np.int64)

    perm = np.argsort(t, kind="stable")
    fs, ts = f[perm], t[perm]
    rnorm = 1.0 / np.sqrt((fs.astype(np.float64) ** 2).sum(1))
    fn = (fs * rnorm[:, None].astype(np.float32)).astype(np.float32)
    fn8 = (fn * S8).astype(f8t)                     # [N, D] fp8 values
    fnT8 = np.ascontiguousarray(fn8.T)              # [D, N]

    in_maps = []
    for c in range(NCORES):
        roll = np.roll(fnT8, 512 - c * 512, axis=1)     # local col l = global (c*512-512+l) % N
        a = roll.reshape(2, 2, 128, N)                  # [kp, i, p, l]
        im = {}
        for kp in range(2):
            for piece in range(2):
                im[f"fnq{kp}{piece}"] = np.ascontiguousarray(
                    a[kp, :, :, piece * 1024:(piece + 1) * 1024].transpose(1, 0, 2))
            im[f"fn{kp}1"] = np.ascontiguousarray(
                a[kp, :, :, 2048:4096].transpose(1, 0, 2))
        # band masks, local band cols of row tile it: global (R0 - PAD + j) % N
        it_i = np.arange(IT)
        R0 = c * 512 + it_i * 128
        rows = R0[:, None] + np.arange(128)[None, :]            # [IT, p]
        g = (R0[:, None] - PAD + np.arange(W)[None, :]) % N     # [IT, j]
        m1 = (ts[rows][:, :, None] == ts[g][:, None, :])        # [IT, p, j]
        im["m1"] = np.ascontiguousarray(
            m1.transpose(1, 0, 2).reshape(128, IT * W).astype(f8t))
        in_maps.append(im)
    return (fn8, ts), in_maps


def _band_covered(ts):
    """Every same-class pair must fall inside the band (guaranteed for any
    remotely Poisson-like class distribution; checked for safety)."""
    cls, counts = np.unique(ts, return_counts=True)
    starts = np.zeros(len(cls) + 1, np.int64)
    starts[1:] = np.cumsum(counts)
    idx = np.searchsorted(cls, ts)
    row_lo, row_hi = starts[idx], starts[idx] + counts[idx]
    R0 = (np.arange(N) // 128) * 128
    return bool(((row_lo >= R0 - PAD) & (row_hi <= R0 - PAD + W)).all())


def _host_post(fn8, ts, lnsum_rows, neg_rows):
    cls, counts = np.unique(ts, return_counts=True)
    idx = np.searchsorted(cls, ts)
    p = counts[idx].astype(np.float64)
    Eii = np.exp((fn8.astype(np.float64) ** 2).sum(1) / (TAU * S8 * S8))
    A = (lnsum_rows - (W - p) * np.log(neg_rows)
         - np.log(Eii + neg_rows))
    fnq = fn8.astype(np.float64) / S8
    g = np.zeros((len(cls), D), np.float64)
    np.add.at(g, idx, fnq)
    B = ((fnq * g[idx]).sum(1) - (fnq ** 2).sum(1)) / TAU
    loss = ((A - B) / p).sum() / p.sum()
    return np.float32(loss)


def _rows_from_out(per_core_outs):
    lnsum = np.empty(N, np.float64)
    neg = np.empty(N, np.float64)
    for c, out in enumerate(per_core_outs):
        ln = np.asarray(out["ln_out"], np.float64)     # [128, IT]
        ng = np.asarray(out["neg_out"], np.float64)
        lnsum[c * ROWS:(c + 1) * ROWS] = ln.T.reshape(ROWS)
        neg[c * ROWS:(c + 1) * ROWS] = ng.T.reshape(ROWS)
    return lnsum, neg


def _run(in_maps, trace=False):
    from concourse.bass_utils import run_bass_kernel_spmd
    nc = _get_nc()
    return run_bass_kernel_spmd(
        nc, in_maps, core_ids=list(range(NCORES)), trace=trace,
    )


def _numpy_fallback(features, targets):
    f = np.asarray(features, np.float64)
    t = np.asarray(targets).astype(np.int64)
    sim = f @ f.T
    nrm = np.sqrt((f ** 2).sum(1))
    nm = np.maximum(nrm[:, None] * nrm[None, :], 1e-8)
    E = np.exp(sim / nm / TAU)
    mask = (t[None, :] == t[:, None])
    np.fill_diagonal(E, 0.0)
    negv = (E * ~mask).sum(1)
    p = mask.sum(1).astype(np.float64)
    with np.errstate(divide="ignore"):
        lm = np.where(mask & (E > 0), np.log(E / (E + negv[:, None])), 0.0)
    return np.float32(-(lm / p[:, None]).sum() / p.sum())


def kernel(features, targets):
    (fn8, ts), in_maps = _host_prep(features, targets)
    if not _band_covered(ts):
        return _numpy_fallback(features, targets)
    res = _run(in_maps, trace=False)
    lnsum_rows, neg_rows = _rows_from_out(res.results)
    return _host_post(fn8, ts, lnsum_rows, neg_rows)
